# revision 1
# baseline (speedup 1.0000x reference)
"""MultiHeadAttention (B=4, S=2048, D=1024, H=16, causal + key mask) on 8 trn2 cores.

Sharding: Megatron-style tensor parallel over heads. Each core owns 2 heads:
column slices of Wq/Wk/Wv (D x 128), the matching row slice of Wp (128 x D).
Each core computes a partial output y_c = attn_c @ Wp_c; host sums the 8
partials and adds bp.

Per-core kernel (all matmuls float32r: full PE rate at N=512, ~2e-4 rel err):
  - x^T [D, B*S] streamed in chunks; projections produce Q^T/K^T
    [128 = 2 heads x 64, B, S] directly (W slice as lhsT, x^T as rhs).
  - V via PE transpose into [s, hd] layout + a ones column so the PV matmul
    also accumulates the softmax denominator (row 64 of the PV psum).
  - Scores computed transposed: S^T[k, q] = K^T_slice.T @ Q^T_slice (K=64).
    Both heads' score blocks land in one [128,1024] 2-bank PSUM tile ->
    ONE additive causal mask (DVE) + ONE exp (ScalarE, key-mask as
    per-partition bias) per k-block. No max-subtraction (logits are O(1)).
  - PV accumulates attnT[hd, q]; psum copied to SBUF fast (frees the bank),
    reciprocal (DVE) -> partition-broadcast (DMA) -> normalize into a
    dedicated attnT buffer (reuses the x-stream pool's SBUF space).
  - Output projection is emitted two groups behind so the normalize chain
    never head-of-line blocks the in-order PE queue.
"""

import numpy as np

P = 128
B, S, D, H = 4, 2048, 1024, 16
HD = D // H  # 64
NCORES = 8
HPC = H // NCORES  # 2 heads per core
BS = B * S  # 8192
NB = S // P  # 16 k-blocks per batch
NG = S // 512  # 4 q-groups per batch

_CACHE = {}


def _build_nc():
    import concourse.mybir as mybir
    from concourse import bacc
    from concourse.tile import TileContext
    from concourse.masks import make_identity
    from contextlib import ExitStack

    f32 = mybir.dt.float32
    f32r = mybir.dt.float32r
    AF = mybir.ActivationFunctionType

    nc = bacc.Bacc("TRN2", target_bir_lowering=False, debug=False,
                   num_devices=NCORES)

    xT_d = nc.dram_tensor("xT", [D, BS], f32r, kind="ExternalInput").ap()
    wq_d = nc.dram_tensor("wq", [D, P], f32r, kind="ExternalInput").ap()
    wk_d = nc.dram_tensor("wk", [D, P], f32r, kind="ExternalInput").ap()
    wv_d = nc.dram_tensor("wv", [D, P], f32r, kind="ExternalInput").ap()
    bq_d = nc.dram_tensor("bq", [P, 1], f32, kind="ExternalInput").ap()
    bk_d = nc.dram_tensor("bk", [P, 1], f32, kind="ExternalInput").ap()
    bv_d = nc.dram_tensor("bv", [P, 1], f32, kind="ExternalInput").ap()
    wp_d = nc.dram_tensor("wp", [P, D], f32r, kind="ExternalInput").ap()
    mb_d = nc.dram_tensor("maskb", [P, B * NB], f32, kind="ExternalInput").ap()
    cm_d = nc.dram_tensor("cmask", [P, 4, 1024], f32,
                          kind="ExternalInput").ap()
    yp_d = nc.dram_tensor("yp", [BS, D], f32, kind="ExternalOutput").ap()

    xT_r = xT_d.rearrange("(o p) n -> p o n", p=P)  # [128, 8, 8192]
    KD = D // P  # 8 contraction chunks

    with TileContext(nc) as tc:
        with ExitStack() as ctx:
            consts = ctx.enter_context(tc.tile_pool(name="consts", bufs=1))
            big = ctx.enter_context(tc.tile_pool(name="big", bufs=1))
            ptpool = ctx.enter_context(tc.tile_pool(name="ptpool", bufs=3))
            npool = ctx.enter_context(tc.tile_pool(name="npool", bufs=2))
            ypool = ctx.enter_context(tc.tile_pool(name="ypool", bufs=3))
            psum = ctx.enter_context(
                tc.tile_pool(name="psum", bufs=2, space="PSUM"))
            sc2pool = ctx.enter_context(
                tc.tile_pool(name="sc2pool", bufs=2, space="PSUM"))
            pvpool = ctx.enter_context(
                tc.tile_pool(name="pvpool", bufs=2, space="PSUM"))

            # ---- constants ----
            wq_sb = consts.tile([P, KD, P], f32r, tag="wq")
            wk_sb = consts.tile([P, KD, P], f32r, tag="wk")
            wv_sb = consts.tile([P, KD, P], f32r, tag="wv")
            nc.sync.dma_start(wq_sb[:], wq_d.rearrange("(o p) m -> p o m", p=P))
            nc.sync.dma_start(wk_sb[:], wk_d.rearrange("(o p) m -> p o m", p=P))
            nc.sync.dma_start(wv_sb[:], wv_d.rearrange("(o p) m -> p o m", p=P))
            wp_sb = consts.tile([P, D], f32r, tag="wp")
            nc.sync.dma_start(wp_sb[:], wp_d)
            bq_sb = consts.tile([P, 1], f32, tag="bq")
            bk_sb = consts.tile([P, 1], f32, tag="bk")
            bv_sb = consts.tile([P, 1], f32, tag="bv")
            nc.sync.dma_start(bq_sb[:], bq_d)
            nc.sync.dma_start(bk_sb[:], bk_d)
            nc.sync.dma_start(bv_sb[:], bv_d)
            mb_sb = consts.tile([P, B * NB], f32, tag="mb")
            nc.sync.dma_start(mb_sb[:], mb_d)
            cm_sb = consts.tile([P, 4, 1024], f32, tag="cm")
            nc.sync.dma_start(cm_sb[:], cm_d)
            ident = consts.tile([P, P], f32, tag="ident")
            make_identity(nc, ident[:])

            # ---- persistent activations ----
            qt_sb = big.tile([P, B, S], f32r, tag="qt")  # Q^T
            kt_sb = big.tile([P, B, S], f32r, tag="kt")  # K^T
            # V in [s, hd] layout + ones col: [p=s%128, h, b, sblock, 65]
            v_sb = big.tile([P, HPC, B, NB, HD + 1], f32r, tag="v")
            nc.vector.memset(v_sb[:, :, :, :, HD].bitcast(f32), 1.0)

            # ---- phase 1: projections (x-stream pools scoped here) ----
            with tc.tile_pool(name="xpool", bufs=2) as xpool, \
                 tc.tile_pool(name="vtpool", bufs=2) as vtpool:
                for c in range(BS // 512):  # 16 chunks of 512 rows, b-major
                    b, sc = divmod(c, NG)
                    xt = xpool.tile([P, KD, 512], f32r, tag="xt")
                    nc.sync.dma_start(xt[:], xT_r[:, :, c * 512:(c + 1) * 512])
                    ssl = slice(sc * 512, (sc + 1) * 512)

                    for which in range(3):
                        w_sb = (wq_sb, wk_sb, wv_sb)[which]
                        ps = psum.tile([P, 512], f32, tag="ps")
                        for o in range(KD):
                            nc.tensor.matmul(
                                ps[:], lhsT=w_sb[:, o, :], rhs=xt[:, o, :],
                                start=(o == 0), stop=(o == KD - 1))
                        if which == 0:
                            nc.scalar.activation(qt_sb[:, b, ssl], ps[:],
                                                 AF.Identity, bias=bq_sb[:])
                        elif which == 1:
                            nc.scalar.activation(kt_sb[:, b, ssl], ps[:],
                                                 AF.Identity, bias=bk_sb[:])
                        else:
                            vt = vtpool.tile([P, 512], f32, tag="vt")
                            nc.scalar.activation(vt[:], ps[:], AF.Identity,
                                                 bias=bv_sb[:])
                            for t in range(4):
                                trp = psum.tile([P, 512], f32, tag="ps")
                                nc.tensor.transpose(
                                    trp[:, :P], vt[:, t * P:(t + 1) * P],
                                    ident[:])
                                sb_i = sc * 4 + t
                                nc.vector.tensor_copy(
                                    v_sb[:, 0, b, sb_i, 0:HD], trp[:, 0:HD])
                                nc.vector.tensor_copy(
                                    v_sb[:, 1, b, sb_i, 0:HD],
                                    trp[:, HD:2 * HD])

            # attnT buffer (reuses the closed x-stream pools' SBUF space)
            atpool = ctx.enter_context(tc.tile_pool(name="atpool", bufs=1))
            at_sb = atpool.tile([P, B, S], f32r, tag="at")

            # ---- phase 2: attention + output projection ----
            def outproj(b, g):
                for qc in range(4):
                    q0 = g * 512 + qc * P
                    r0 = b * S + q0
                    y_sb = ypool.tile([P, D], f32, tag="y",
                                      name=f"y_{b}_{g}_{qc}")
                    for half in range(2):
                        yp_ps = psum.tile([P, 512], f32, tag="ps",
                                          name=f"yps_{b}_{g}_{qc}_{half}")
                        nc.tensor.matmul(
                            yp_ps[:],
                            lhsT=at_sb[:, b, q0:q0 + P],
                            rhs=wp_sb[:, half * 512:(half + 1) * 512],
                            start=True, stop=True)
                        ysl = y_sb[:, half * 512:(half + 1) * 512]
                        nc.scalar.activation(ysl, yp_ps[:], AF.Copy)
                    nc.sync.dma_start(yp_d[r0:r0 + P, :], y_sb[:])

            pending = []
            for b in range(B):
                for g in range(NG):
                    gsl = slice(g * 512, (g + 1) * 512)
                    nkb = 4 * (g + 1)
                    pvs = [pvpool.tile([P, 512], f32, tag="pv",
                                       name=f"pv_{b}_{g}_{h}")
                           for h in range(HPC)]
                    for kb in range(nkb):
                        j = kb - 4 * g
                        col = b * NB + kb
                        # deep-diagonal blocks (j>=2): q < 128*j is fully
                        # masked; restrict to q in [256,512) (N=256 keeps
                        # full f32r rate; contiguous APs only)
                        qo = 256 if j >= 2 else 0
                        sc2 = sc2pool.tile([P, 1024], f32, tag="sc2",
                                           name=f"sc2_{b}_{g}_{kb}")
                        for h in range(HPC):
                            hsl = slice(h * HD, (h + 1) * HD)
                            nc.tensor.matmul(
                                sc2[:, h * 512 + qo:(h + 1) * 512],
                                lhsT=kt_sb[hsl, b, kb * P:(kb + 1) * P],
                                rhs=qt_sb[hsl, b,
                                          g * 512 + qo:(g + 1) * 512],
                                start=True, stop=True)
                        pt = ptpool.tile([P, 1024], f32r, tag="pt")
                        if qo == 0:
                            if j >= 0:  # diagonal: additive causal mask
                                nc.vector.tensor_add(sc2[:], sc2[:],
                                                     cm_sb[:, j, :])
                            nc.scalar.activation(pt[:], sc2[:], AF.Exp,
                                                 bias=mb_sb[:, col:col + 1])
                        else:
                            for h in range(HPC):
                                hs = slice(h * 512 + qo, (h + 1) * 512)
                                nc.vector.tensor_add(sc2[:, hs], sc2[:, hs],
                                                     cm_sb[:, j, hs])
                                nc.scalar.activation(
                                    pt[:, hs], sc2[:, hs], AF.Exp,
                                    bias=mb_sb[:, col:col + 1])
                        for h in range(HPC):
                            nc.tensor.matmul(
                                pvs[h][0:HD + 1, qo:512],
                                lhsT=v_sb[:, h, b, kb, :],
                                rhs=pt[:, h * 512 + qo:(h + 1) * 512],
                                start=(kb == 0), stop=(kb == nkb - 1))
                    if len(pending) >= 2:
                        outproj(*pending.pop(0))
                    pending.append((b, g))
                    for h in range(HPC):
                        # free the pv psum slot fast: copy [65,512] to SBUF
                        pvs_sb = npool.tile([P, 512], f32, tag="pvs")
                        nc.scalar.activation(pvs_sb[0:HD + 1, :],
                                             pvs[h][0:HD + 1, :], AF.Copy)
                        # 1/sum(exp) (row 64), broadcast to 64 partitions
                        rec = npool.tile([P, 512], f32, tag="rec")
                        nc.vector.reciprocal(
                            rec[HD:HD + 1, :], pvs_sb[HD:HD + 1, :])
                        sx = npool.tile([HD, 512], f32, tag="sx")
                        nc.sync.dma_start(
                            sx[:],
                            rec[HD:HD + 1, None, :]
                            .to_broadcast((1, HD, 512)))
                        if h == 0:
                            nc.vector.tensor_mul(
                                at_sb[0:HD, b, gsl], pvs_sb[0:HD, :], sx[:])
                        else:
                            tmp = npool.tile([HD, 512], f32r, tag="tmp")
                            nc.vector.tensor_mul(
                                tmp[:], pvs_sb[0:HD, :], sx[:])
                            nc.sync.dma_start(at_sb[HD:2 * HD, b, gsl],
                                              tmp[:])

            for pg in pending:
                outproj(*pg)

    nc.compile()
    return nc


def _get_nc():
    if "nc" not in _CACHE:
        _CACHE["nc"] = _build_nc()
    return _CACHE["nc"]


def make_in_maps(x, attention_mask, Wq, bq, Wk, bk, Wv, bv, Wp, bp):
    """Host-side sharding: build the 8 per-core device input maps."""
    x = np.asarray(x, dtype=np.float32)
    scale = np.float32(1.0 / np.sqrt(HD))
    xT = np.ascontiguousarray(x.reshape(BS, D).T)  # [D, BS]
    mb = (np.asarray(attention_mask).astype(np.float32) - 1.0) * np.float32(1e9)
    mb = np.ascontiguousarray(
        mb.reshape(B, NB, P).transpose(2, 0, 1).reshape(P, B * NB))
    # causal diag masks (additive): 0 where 128*j + p <= f, else -1e9;
    # duplicated for the two head halves of the [128,1024] scores tile.
    pp = np.arange(P)[:, None]
    ff = np.arange(512)[None, :]
    cm = np.stack(
        [np.where(P * j + pp <= ff, 0.0, -1e9).astype(np.float32)
         for j in range(4)], axis=1)  # [128, 4, 512]
    cm = np.ascontiguousarray(np.concatenate([cm, cm], axis=-1))

    Wq = np.asarray(Wq, np.float32) * scale
    bq = np.asarray(bq, np.float32) * scale
    Wk = np.asarray(Wk, np.float32)
    bk = np.asarray(bk, np.float32)
    Wv = np.asarray(Wv, np.float32)
    bv = np.asarray(bv, np.float32)
    Wp = np.asarray(Wp, np.float32)

    in_maps = []
    for c in range(NCORES):
        cs = slice(c * P, (c + 1) * P)
        in_maps.append({
            "xT": xT,
            "wq": np.ascontiguousarray(Wq[:, cs]),
            "wk": np.ascontiguousarray(Wk[:, cs]),
            "wv": np.ascontiguousarray(Wv[:, cs]),
            "bq": np.ascontiguousarray(bq[cs].reshape(P, 1)),
            "bk": np.ascontiguousarray(bk[cs].reshape(P, 1)),
            "bv": np.ascontiguousarray(bv[cs].reshape(P, 1)),
            "wp": np.ascontiguousarray(Wp[cs, :]),
            "maskb": mb,
            "cmask": cm,
        })
    return in_maps


def run(inputs, trace=False, tmpdir=None):
    """Compile (cached) + run on 8 cores. Returns (output, BassKernelResults)."""
    from concourse import bass_utils
    nc = _get_nc()
    in_maps = make_in_maps(**inputs)
    kwargs = {}
    if trace:
        kwargs = dict(trace=True, tmpdir=tmpdir)
    res = bass_utils.run_bass_kernel_spmd(
        nc, in_maps, core_ids=list(range(NCORES)), **kwargs)
    acc = np.zeros((BS, D), dtype=np.float64)
    for r in res.results:
        acc += r["yp"].astype(np.float64)
    out = (acc + np.asarray(inputs["bp"], np.float64)[None, :]).astype(
        np.float32)
    return out.reshape(B, S, D), res


def kernel(**inputs) -> np.ndarray:
    out, _ = run(inputs, trace=False)
    return out



# revision 20
# speedup vs baseline: 1.2777x; 1.2777x over previous
"""MultiHeadAttention (B=4, S=2048, D=1024, H=16, causal + key mask) on 8 trn2 cores.

Sharding: Megatron-style tensor parallel over heads. Each core owns 2 heads:
column slices of Wq/Wk/Wv (D x 128), the matching row slice of Wp (128 x D).
Each core computes a partial output y_c = attn_c @ Wp_c; host sums the 8
partials (bf16) and adds bp.

v2 redesign vs the first working kernel (551us):
  - bf16 operands everywhere (psums stay f32): halves x-read / y-write DMA,
    full PE rate at any free size (the old f32r needed N>=256).
  - Exact causal skipping: k-block (b,g,kb) computes only q in [128j, 512)
    (j = kb-4g), so exp/mask work drops to the 136-tile lower triangle, and
    the causal mask is ONE [128,128] add on the true diagonal tile only
    (the same pattern every time -> a single constant, 2-head strided op).
  - The 3.3us-per-call DVE RECIPROCAL -> reciprocal_approx_fast custom op.
  - PV psum is read directly by the normalize mul (no ACT copy); outproj
    psum->sbuf casts run on DVE, freeing ACT for exp only.
  - PE order: PV(kb) is emitted after scores(kb+3) so the PE never waits on
    the ACT exp; outproj (2 groups behind) is interleaved between score
    blocks. Keeps the PE warm (HAM K=8/8) through phase 2.
"""

import numpy as np

P = 128
B, S, D, H = 4, 2048, 1024, 16
HD = D // H  # 64
NCORES = 8
BS = B * S  # 8192
NB = S // P  # 16 k-blocks per batch
NG = S // 512  # 4 q-groups per batch

_CACHE = {}
DEBUG = False


def _build_nc():
    import concourse.mybir as mybir
    from concourse import bacc
    from concourse.tile import TileContext
    from concourse.masks import make_identity
    from contextlib import ExitStack

    f32 = mybir.dt.float32
    bf16 = mybir.dt.bfloat16
    AF = mybir.ActivationFunctionType

    nc = bacc.Bacc("TRN2", target_bir_lowering=False, debug=False,
                   num_devices=NCORES)

    xT_d = nc.dram_tensor("xT", [D, BS], bf16, kind="ExternalInput").ap()
    wq_d = nc.dram_tensor("wq", [D, P], bf16, kind="ExternalInput").ap()
    wk_d = nc.dram_tensor("wk", [D, P], bf16, kind="ExternalInput").ap()
    wv_d = nc.dram_tensor("wv", [D, P], bf16, kind="ExternalInput").ap()
    bq_d = nc.dram_tensor("bq", [P, 1], f32, kind="ExternalInput").ap()
    bk_d = nc.dram_tensor("bk", [P, 1], f32, kind="ExternalInput").ap()
    bv_d = nc.dram_tensor("bv", [P, 1], f32, kind="ExternalInput").ap()
    wp_d = nc.dram_tensor("wp", [P, D], bf16, kind="ExternalInput").ap()
    mb_d = nc.dram_tensor("maskb", [P, B * NB], f32, kind="ExternalInput").ap()
    cm_d = nc.dram_tensor("cmask", [P, 2, P], f32, kind="ExternalInput").ap()
    yp_d = nc.dram_tensor("yp", [BS, D], bf16, kind="ExternalOutput").ap()
    if DEBUG:
        dq_d = nc.dram_tensor("dq", [P, 512], bf16,
                              kind="ExternalOutput").ap()
        dk_d = nc.dram_tensor("dk", [P, 512], bf16,
                              kind="ExternalOutput").ap()
        dv_d = nc.dram_tensor("dv", [P, 2, 4, HD + 1], bf16,
                              kind="ExternalOutput").ap()
        da_d = nc.dram_tensor("da", [P, S], bf16,
                              kind="ExternalOutput").ap()
        dr_d = nc.dram_tensor("dr", [1, 2, 512], f32,
                              kind="ExternalOutput").ap()
        dp_d = nc.dram_tensor("dp", [P, 512], f32,
                              kind="ExternalOutput").ap()

    xT_r = xT_d.rearrange("(o p) n -> p o n", p=P)  # [128, 8, 8192]
    KD = D // P  # 8 contraction chunks

    with TileContext(nc) as tc:
        with ExitStack() as ctx:
            consts = ctx.enter_context(tc.tile_pool(name="consts", bufs=1))
            big = ctx.enter_context(tc.tile_pool(name="big", bufs=1))
            ptpool = ctx.enter_context(tc.tile_pool(name="ptpool", bufs=4))
            recpool = ctx.enter_context(tc.tile_pool(name="recpool", bufs=2))
            pvsbpool = ctx.enter_context(tc.tile_pool(name="pvsbpool", bufs=4))
            sxpool = ctx.enter_context(tc.tile_pool(name="sxpool", bufs=4))
            ypool = ctx.enter_context(tc.tile_pool(name="ypool", bufs=3))
            # one shared 3-deep ring of [128, 2, 512] f32 slots (6 banks):
            # phase-1 proj psums + V-transpose, phase-2 score tiles + outproj
            # accumulators all rotate through it. pv gets the other 2 banks.
            mnpool = ctx.enter_context(
                tc.tile_pool(name="mnpool", bufs=3, space="PSUM"))
            pvpool = ctx.enter_context(
                tc.tile_pool(name="pvpool", bufs=2, space="PSUM"))

            # ---- constants ----
            wq_sb = consts.tile([P, KD, P], bf16, tag="wq")
            wk_sb = consts.tile([P, KD, P], bf16, tag="wk")
            wv_sb = consts.tile([P, KD, P], bf16, tag="wv")
            nc.sync.dma_start(wq_sb[:], wq_d.rearrange("(o p) m -> p o m", p=P))
            nc.sync.dma_start(wk_sb[:], wk_d.rearrange("(o p) m -> p o m", p=P))
            nc.sync.dma_start(wv_sb[:], wv_d.rearrange("(o p) m -> p o m", p=P))
            wp_sb = consts.tile([P, D], bf16, tag="wp")
            nc.sync.dma_start(wp_sb[:], wp_d)
            bq_sb = consts.tile([P, 1], f32, tag="bq")
            bk_sb = consts.tile([P, 1], f32, tag="bk")
            bv_sb = consts.tile([P, 1], f32, tag="bv")
            nc.sync.dma_start(bq_sb[:], bq_d)
            nc.sync.dma_start(bk_sb[:], bk_d)
            nc.sync.dma_start(bv_sb[:], bv_d)
            mb_sb = consts.tile([P, B * NB], f32, tag="mb")
            nc.sync.dma_start(mb_sb[:], mb_d)
            cm_sb = consts.tile([P, 2, P], f32, tag="cm")
            nc.sync.dma_start(cm_sb[:], cm_d)
            ident = consts.tile([P, P], bf16, tag="ident")
            make_identity(nc, ident[:])

            # ---- persistent activations ----
            qt_sb = big.tile([P, B, S], bf16, tag="qt")  # Q^T
            kt_sb = big.tile([P, B, S], bf16, tag="kt")  # K^T
            # V in [s, hd] layout + ones col: [p=s%128, h, b, sblock, 65]
            v_sb = big.tile([P, 2, B, NB, HD + 1], bf16, tag="v")
            nc.vector.memset(v_sb[:, :, :, :, HD:HD + 1], 1.0)
            at_sb = big.tile([P, B, S], bf16, tag="at")  # attn^T (normalized)

            # ---- phase 1: projections ----
            with tc.tile_pool(name="xpool", bufs=2) as xpool, \
                 tc.tile_pool(name="vtpool", bufs=2) as vtpool:
                for c in range(BS // 512):  # 16 chunks of 512 rows, b-major
                    b, sc = divmod(c, NG)
                    xt = xpool.tile([P, KD, 512], bf16, tag="xt")
                    nc.sync.dma_start(xt[:], xT_r[:, :, c * 512:(c + 1) * 512])
                    ssl = slice(sc * 512, (sc + 1) * 512)

                    for which in range(3):
                        w_sb = (wq_sb, wk_sb, wv_sb)[which]
                        ps = mnpool.tile([P, 512], f32, tag="mn")
                        for o in range(KD):
                            nc.tensor.matmul(
                                ps[:], lhsT=w_sb[:, o, :], rhs=xt[:, o, :],
                                start=(o == 0), stop=(o == KD - 1))
                        if which == 0:
                            nc.scalar.activation(qt_sb[:, b, ssl], ps[:],
                                                 AF.Identity, bias=bq_sb[:])
                        elif which == 1:
                            nc.scalar.activation(kt_sb[:, b, ssl], ps[:],
                                                 AF.Identity, bias=bk_sb[:])
                        else:
                            vt = vtpool.tile([P, 512], bf16, tag="vt")
                            nc.scalar.activation(vt[:], ps[:], AF.Identity,
                                                 bias=bv_sb[:])
                            trp = mnpool.tile([P, 4, P], bf16, tag="mn",
                                              name=f"trp_{c}")
                            for t in range(4):
                                nc.tensor.transpose(
                                    trp[:, t, :], vt[:, t * P:(t + 1) * P],
                                    ident[:])
                            for h in range(2):
                                nc.vector.tensor_copy(
                                    v_sb[:, h, b, 4 * sc:4 * sc + 4, 0:HD],
                                    trp[:, :, h * HD:(h + 1) * HD])

            # ---- phase 2: attention + output projection ----
            def outproj_jobs(b, g):
                """Yield the 4 per-qtile emitters for group (b,g)."""
                for qc in range(4):
                    def job(qc=qc):
                        q0 = g * 512 + qc * P
                        r0 = b * S + q0
                        y_sb = ypool.tile([P, 2, 512], bf16, tag="y",
                                          name=f"y_{b}_{g}_{qc}")
                        yp = mnpool.tile([P, 2, 512], f32, tag="mn",
                                         name=f"yps_{b}_{g}_{qc}")
                        for half in range(2):
                            nc.tensor.matmul(
                                yp[:, half, :],
                                lhsT=at_sb[:, b, q0:q0 + P],
                                rhs=wp_sb[:, half * 512:(half + 1) * 512],
                                start=True, stop=True)
                        nc.vector.tensor_copy(y_sb[:], yp[:])
                        nc.gpsimd.dma_start(
                            yp_d[r0:r0 + P, :]
                            .rearrange("p (h n) -> p h n", h=2), y_sb[:])
                    yield job

            pv_pending = []   # queued PV emitters (depth 3 behind scores)
            op_pending = []   # outproj job generators, >= 2 groups behind

            def drain_pv(keep):
                while len(pv_pending) > keep:
                    pv_pending.pop(0)()

            def emit_outproj_some(n):
                for _ in range(n):
                    if not op_pending:
                        return
                    try:
                        job = next(op_pending[0])
                    except StopIteration:
                        op_pending.pop(0)
                        continue
                    job()

            op_ready = []  # groups whose normalize is emitted
            for b in range(B):
                for g in range(NG):
                    gsl = slice(g * 512, (g + 1) * 512)
                    nkb = 4 * (g + 1)
                    pvs = [pvpool.tile([P, 512], f32, tag="pv",
                                       name=f"pv_{b}_{g}_{h}")
                           for h in range(2)]
                    for kb in range(nkb):
                        j = kb - 4 * g
                        col = b * NB + kb
                        qo = 128 * j if j > 0 else 0
                        sc2 = mnpool.tile([P, 2, 512], f32, tag="mn",
                                          name=f"sc2_{b}_{g}_{kb}")
                        for h in range(2):
                            hsl = slice(h * HD, (h + 1) * HD)
                            nc.tensor.matmul(
                                sc2[:, h, qo:512],
                                lhsT=kt_sb[hsl, b, kb * P:(kb + 1) * P],
                                rhs=qt_sb[hsl, b,
                                          g * 512 + qo:(g + 1) * 512],
                                start=True, stop=True)
                        pt = ptpool.tile([P, 2, 512], bf16, tag="pt")
                        if j >= 0:  # diagonal tile: additive causal mask
                            nc.vector.tensor_add(
                                sc2[:, :, qo:qo + P], sc2[:, :, qo:qo + P],
                                cm_sb[:])
                        nc.scalar.activation(pt[:, :, qo:512],
                                             sc2[:, :, qo:512], AF.Exp,
                                             bias=mb_sb[:, col:col + 1])

                        def pv_job(kb=kb, qo=qo, pt=pt, pvs=pvs, b=b,
                                   nkb=nkb):
                            for h in range(2):
                                nc.tensor.matmul(
                                    pvs[h][0:HD + 1, qo:512],
                                    lhsT=v_sb[:, h, b, kb, :],
                                    rhs=pt[:, h, qo:512],
                                    start=(kb == 0), stop=(kb == nkb - 1))
                        pv_pending.append(pv_job)
                        drain_pv(3)
                        if kb >= 1:
                            emit_outproj_some(1)

                    # group end: flush queued outproj slack, normalize chain
                    emit_outproj_some(1)

                    def normalize(b=b, g=g, gsl=gsl, pvs=pvs):
                        # free the pv psum banks FAST: one copy per head to
                        # SBUF; everything downstream reads the SBUF copy
                        # (also: reciprocal_approx_fast is wrong on PSUM
                        # inputs - SBUF input is required).
                        pvsb = [pvsbpool.tile([HD + 1, 512], f32, tag="pvsb",
                                              name=f"pvsb_{b}_{g}_{h}")
                                for h in range(2)]
                        for h in range(2):
                            nc.vector.tensor_copy(pvsb[h][:],
                                                  pvs[h][0:HD + 1, :])
                        # reciprocal_approx_fast only works at partition base
                        # 0: DMA-shift the denom rows (partition HD) down to
                        # partition 0, recip there, then broadcast.
                        dn = recpool.tile([1, 2, 512], f32, tag="dn",
                                          name=f"dn_{b}_{g}")
                        for h in range(2):
                            nc.sync.dma_start(dn[:, h, :],
                                              pvsb[h][HD:HD + 1, :])
                        rec = recpool.tile([1, 2, 512], f32, tag="rec",
                                           name=f"rec_{b}_{g}")
                        nc.vector.reciprocal_approx_fast(
                            rec[:, :, :], dn[:, :, :])
                        if DEBUG and b == 0 and g == 0:
                            nc.sync.dma_start(dp_d[0:HD + 1, :], pvsb[0][:])
                            nc.sync.dma_start(dr_d[:], rec[:])
                        for h in range(2):
                            sx = sxpool.tile([HD, 512], f32, tag="sx",
                                             name=f"sx_{b}_{g}_{h}")
                            nc.sync.dma_start(
                                sx[:],
                                rec[:, h, None, :]
                                .to_broadcast((1, HD, 512)))
                            if h == 0:
                                nc.gpsimd.tensor_mul(
                                    at_sb[0:HD, b, gsl],
                                    pvsb[0][0:HD, :], sx[:])
                            else:
                                # head 1 lands on partitions 64..127: engines
                                # cannot shift partitions, so mul to a tmp and
                                # DMA it into place.
                                tmp = sxpool.tile([HD, 512], bf16, tag="tmp",
                                                  name=f"tmp_{b}_{g}")
                                nc.gpsimd.tensor_mul(
                                    tmp[:], pvsb[1][0:HD, :], sx[:])
                                nc.sync.dma_start(
                                    at_sb[HD:2 * HD, b, gsl], tmp[:])
                    # run normalize after the last PV of THIS group emits
                    # (pv_pending holds <=3 jobs incl. this group's tail)
                    pv_pending.append(normalize)
                    op_ready.append((b, g))
                    if len(op_ready) >= 3:
                        op_pending.append(outproj_jobs(*op_ready.pop(0)))

            drain_pv(0)
            for bg in op_ready:
                op_pending.append(outproj_jobs(*bg))
            emit_outproj_some(1000)

            if DEBUG:
                nc.sync.dma_start(dq_d[:], qt_sb[:, 0, 0:512])
                nc.sync.dma_start(dk_d[:], kt_sb[:, 0, 0:512])
                nc.sync.dma_start(dv_d[:], v_sb[:, :, 0, 0:4, :])
                nc.sync.dma_start(da_d[:], at_sb[:, 0, :])

    nc.compile()
    return nc


def _get_nc():
    if "nc" not in _CACHE:
        _CACHE["nc"] = _build_nc()
    return _CACHE["nc"]


def make_in_maps(x, attention_mask, Wq, bq, Wk, bk, Wv, bv, Wp, bp):
    """Host-side sharding: build the 8 per-core device input maps."""
    import ml_dtypes
    bf16 = ml_dtypes.bfloat16
    x = np.asarray(x, dtype=np.float32)
    scale = np.float32(1.0 / np.sqrt(HD))
    xT = np.ascontiguousarray(x.reshape(BS, D).T.astype(bf16))  # [D, BS]
    mb = (np.asarray(attention_mask).astype(np.float32) - 1.0) * np.float32(1e9)
    mb = np.ascontiguousarray(
        mb.reshape(B, NB, P).transpose(2, 0, 1).reshape(P, B * NB))
    # causal diagonal-tile mask (additive): 0 where q_local >= k_local,
    # else -1e9; duplicated for the two heads' strided slices.
    pp = np.arange(P)[:, None]
    ff = np.arange(P)[None, :]
    cm1 = np.where(ff >= pp, 0.0, -1e9).astype(np.float32)
    cm = np.ascontiguousarray(
        np.stack([cm1, cm1], axis=1))  # [128, 2, 128]

    Wq = (np.asarray(Wq, np.float32) * scale).astype(bf16)
    bq = np.asarray(bq, np.float32) * scale
    Wk = np.asarray(Wk, np.float32).astype(bf16)
    bk = np.asarray(bk, np.float32)
    Wv = np.asarray(Wv, np.float32).astype(bf16)
    bv = np.asarray(bv, np.float32)
    Wp = np.asarray(Wp, np.float32).astype(bf16)

    in_maps = []
    for c in range(NCORES):
        cs = slice(c * P, (c + 1) * P)
        in_maps.append({
            "xT": xT,
            "wq": np.ascontiguousarray(Wq[:, cs]),
            "wk": np.ascontiguousarray(Wk[:, cs]),
            "wv": np.ascontiguousarray(Wv[:, cs]),
            "bq": np.ascontiguousarray(bq[cs].reshape(P, 1)),
            "bk": np.ascontiguousarray(bk[cs].reshape(P, 1)),
            "bv": np.ascontiguousarray(bv[cs].reshape(P, 1)),
            "wp": np.ascontiguousarray(Wp[cs, :]),
            "maskb": mb,
            "cmask": cm,
        })
    return in_maps


def run(inputs, trace=False, tmpdir=None):
    """Compile (cached) + run on 8 cores. Returns (output, BassKernelResults)."""
    from concourse import bass_utils
    nc = _get_nc()
    in_maps = make_in_maps(**inputs)
    kwargs = {}
    if trace:
        kwargs = dict(trace=True, tmpdir=tmpdir)
    res = bass_utils.run_bass_kernel_spmd(
        nc, in_maps, core_ids=list(range(NCORES)), **kwargs)
    acc = np.zeros((BS, D), dtype=np.float64)
    for r in res.results:
        acc += r["yp"].astype(np.float64)
    out = (acc + np.asarray(inputs["bp"], np.float64)[None, :]).astype(
        np.float32)
    return out.reshape(B, S, D), res


def kernel(**inputs) -> np.ndarray:
    out, _ = run(inputs, trace=False)
    return out


# revision 26
# speedup vs baseline: 1.2811x; 1.0027x over previous
"""MultiHeadAttention (B=4, S=2048, D=1024, H=16, causal + key mask) on 8 trn2 cores.

Sharding: Megatron-style tensor parallel over heads. Each core owns 2 heads:
column slices of Wq/Wk/Wv (D x 128), the matching row slice of Wp (128 x D).
Each core computes a partial output y_c = attn_c @ Wp_c; host sums the 8
partials (bf16) and adds bp.

v2 redesign vs the first working kernel (551us):
  - bf16 operands everywhere (psums stay f32): halves x-read / y-write DMA,
    full PE rate at any free size (the old f32r needed N>=256).
  - Exact causal skipping: k-block (b,g,kb) computes only q in [128j, 512)
    (j = kb-4g), so exp/mask work drops to the 136-tile lower triangle, and
    the causal mask is ONE [128,128] add on the true diagonal tile only
    (the same pattern every time -> a single constant, 2-head strided op).
  - The 3.3us-per-call DVE RECIPROCAL -> reciprocal_approx_fast custom op.
  - PV psum is read directly by the normalize mul (no ACT copy); outproj
    psum->sbuf casts run on DVE, freeing ACT for exp only.
  - PE order: PV(kb) is emitted after scores(kb+3) so the PE never waits on
    the ACT exp; outproj (2 groups behind) is interleaved between score
    blocks. Keeps the PE warm (HAM K=8/8) through phase 2.
"""

import numpy as np

P = 128
B, S, D, H = 4, 2048, 1024, 16
HD = D // H  # 64
NCORES = 8
BS = B * S  # 8192
NB = S // P  # 16 k-blocks per batch
NG = S // 512  # 4 q-groups per batch

_CACHE = {}
DEBUG = False


def _build_nc():
    import concourse.mybir as mybir
    from concourse import bacc
    from concourse.tile import TileContext
    from concourse.masks import make_identity
    from contextlib import ExitStack

    f32 = mybir.dt.float32
    bf16 = mybir.dt.bfloat16
    AF = mybir.ActivationFunctionType

    nc = bacc.Bacc("TRN2", target_bir_lowering=False, debug=False,
                   num_devices=NCORES)

    xT_d = nc.dram_tensor("xT", [D, BS], bf16, kind="ExternalInput").ap()
    wq_d = nc.dram_tensor("wq", [D, P], bf16, kind="ExternalInput").ap()
    wk_d = nc.dram_tensor("wk", [D, P], bf16, kind="ExternalInput").ap()
    wv_d = nc.dram_tensor("wv", [D, P], bf16, kind="ExternalInput").ap()
    bq_d = nc.dram_tensor("bq", [P, 1], f32, kind="ExternalInput").ap()
    bk_d = nc.dram_tensor("bk", [P, 1], f32, kind="ExternalInput").ap()
    bv_d = nc.dram_tensor("bv", [P, 1], f32, kind="ExternalInput").ap()
    wp_d = nc.dram_tensor("wp", [P, D], bf16, kind="ExternalInput").ap()
    mb_d = nc.dram_tensor("maskb", [P, B * NB], f32, kind="ExternalInput").ap()
    cm_d = nc.dram_tensor("cmask", [P, 2, P], f32, kind="ExternalInput").ap()
    yp_d = nc.dram_tensor("yp", [BS, D], bf16, kind="ExternalOutput").ap()
    if DEBUG:
        dq_d = nc.dram_tensor("dq", [P, 512], bf16,
                              kind="ExternalOutput").ap()
        dk_d = nc.dram_tensor("dk", [P, 512], bf16,
                              kind="ExternalOutput").ap()
        dv_d = nc.dram_tensor("dv", [P, 2, 4, HD + 1], bf16,
                              kind="ExternalOutput").ap()
        da_d = nc.dram_tensor("da", [P, S], bf16,
                              kind="ExternalOutput").ap()
        dr_d = nc.dram_tensor("dr", [1, 2, 512], f32,
                              kind="ExternalOutput").ap()
        dp_d = nc.dram_tensor("dp", [P, 512], f32,
                              kind="ExternalOutput").ap()

    xT_r = xT_d.rearrange("(o p) n -> p o n", p=P)  # [128, 8, 8192]
    KD = D // P  # 8 contraction chunks

    with TileContext(nc) as tc:
        with ExitStack() as ctx:
            consts = ctx.enter_context(tc.tile_pool(name="consts", bufs=1))
            big = ctx.enter_context(tc.tile_pool(name="big", bufs=1))
            ptpool = ctx.enter_context(tc.tile_pool(name="ptpool", bufs=4))
            recpool = ctx.enter_context(tc.tile_pool(name="recpool", bufs=3))
            pvsbpool = ctx.enter_context(tc.tile_pool(name="pvsbpool", bufs=6))
            sxpool = ctx.enter_context(tc.tile_pool(name="sxpool", bufs=6))
            ypool = ctx.enter_context(tc.tile_pool(name="ypool", bufs=3))
            # one shared 3-deep ring of [128, 2, 512] f32 slots (6 banks):
            # phase-1 proj psums + V-transpose, phase-2 score tiles + outproj
            # accumulators all rotate through it. pv gets the other 2 banks.
            mnpool = ctx.enter_context(
                tc.tile_pool(name="mnpool", bufs=3, space="PSUM"))
            pvpool = ctx.enter_context(
                tc.tile_pool(name="pvpool", bufs=2, space="PSUM"))

            # ---- constants ----
            wq_sb = consts.tile([P, KD, P], bf16, tag="wq")
            wk_sb = consts.tile([P, KD, P], bf16, tag="wk")
            wv_sb = consts.tile([P, KD, P], bf16, tag="wv")
            nc.sync.dma_start(wq_sb[:], wq_d.rearrange("(o p) m -> p o m", p=P))
            nc.sync.dma_start(wk_sb[:], wk_d.rearrange("(o p) m -> p o m", p=P))
            nc.sync.dma_start(wv_sb[:], wv_d.rearrange("(o p) m -> p o m", p=P))
            wp_sb = consts.tile([P, D], bf16, tag="wp")
            nc.sync.dma_start(wp_sb[:], wp_d)
            bq_sb = consts.tile([P, 1], f32, tag="bq")
            bk_sb = consts.tile([P, 1], f32, tag="bk")
            bv_sb = consts.tile([P, 1], f32, tag="bv")
            nc.sync.dma_start(bq_sb[:], bq_d)
            nc.sync.dma_start(bk_sb[:], bk_d)
            nc.sync.dma_start(bv_sb[:], bv_d)
            mb_sb = consts.tile([P, B * NB], f32, tag="mb")
            nc.sync.dma_start(mb_sb[:], mb_d)
            cm_sb = consts.tile([P, 2, P], f32, tag="cm")
            nc.sync.dma_start(cm_sb[:], cm_d)
            ident = consts.tile([P, P], bf16, tag="ident")
            make_identity(nc, ident[:])

            # ---- persistent activations ----
            qt_sb = big.tile([P, B, S], bf16, tag="qt")  # Q^T
            kt_sb = big.tile([P, B, S], bf16, tag="kt")  # K^T
            # V in [s, hd] layout + ones col: [p=s%128, h, b, sblock, 65]
            v_sb = big.tile([P, 2, B, NB, HD + 1], bf16, tag="v")
            nc.vector.memset(v_sb[:, :, :, :, HD:HD + 1], 1.0)
            at_sb = big.tile([P, B, S], bf16, tag="at")  # attn^T (normalized)

            # ---- phase 1: projections ----
            with tc.tile_pool(name="xpool", bufs=2) as xpool, \
                 tc.tile_pool(name="vtpool", bufs=2) as vtpool:
                for c in range(BS // 512):  # 16 chunks of 512 rows, b-major
                    b, sc = divmod(c, NG)
                    xt = xpool.tile([P, KD, 512], bf16, tag="xt")
                    nc.sync.dma_start(xt[:], xT_r[:, :, c * 512:(c + 1) * 512])
                    ssl = slice(sc * 512, (sc + 1) * 512)

                    for which in range(3):
                        w_sb = (wq_sb, wk_sb, wv_sb)[which]
                        ps = mnpool.tile([P, 512], f32, tag="mn")
                        for o in range(KD):
                            nc.tensor.matmul(
                                ps[:], lhsT=w_sb[:, o, :], rhs=xt[:, o, :],
                                start=(o == 0), stop=(o == KD - 1))
                        if which == 0:
                            nc.scalar.activation(qt_sb[:, b, ssl], ps[:],
                                                 AF.Identity, bias=bq_sb[:])
                        elif which == 1:
                            nc.scalar.activation(kt_sb[:, b, ssl], ps[:],
                                                 AF.Identity, bias=bk_sb[:])
                        else:
                            vt = vtpool.tile([P, 512], bf16, tag="vt")
                            nc.scalar.activation(vt[:], ps[:], AF.Identity,
                                                 bias=bv_sb[:])
                            trp = mnpool.tile([P, 4, P], bf16, tag="mn",
                                              name=f"trp_{c}")
                            for t in range(4):
                                nc.tensor.transpose(
                                    trp[:, t, :], vt[:, t * P:(t + 1) * P],
                                    ident[:])
                            for h in range(2):
                                nc.vector.tensor_copy(
                                    v_sb[:, h, b, 4 * sc:4 * sc + 4, 0:HD],
                                    trp[:, :, h * HD:(h + 1) * HD])

            # ---- phase 2: attention + output projection ----
            def outproj_jobs(b, g):
                """Yield the 4 per-qtile emitters for group (b,g)."""
                for qc in range(4):
                    def job(qc=qc):
                        q0 = g * 512 + qc * P
                        r0 = b * S + q0
                        y_sb = ypool.tile([P, 2, 512], bf16, tag="y",
                                          name=f"y_{b}_{g}_{qc}")
                        yp = mnpool.tile([P, 2, 512], f32, tag="mn",
                                         name=f"yps_{b}_{g}_{qc}")
                        for half in range(2):
                            nc.tensor.matmul(
                                yp[:, half, :],
                                lhsT=at_sb[:, b, q0:q0 + P],
                                rhs=wp_sb[:, half * 512:(half + 1) * 512],
                                start=True, stop=True)
                        nc.vector.tensor_copy(y_sb[:], yp[:])
                        nc.gpsimd.dma_start(
                            yp_d[r0:r0 + P, :]
                            .rearrange("p (h n) -> p h n", h=2), y_sb[:])
                    yield job

            pv_pending = []   # queued PV emitters (depth 3 behind scores)
            op_pending = []   # outproj job generators, >= 3 groups behind
            stage_q = []      # (stageB, stageC) of the previous group
            stageC_q = []     # stageC jobs, two groups behind

            def drain_pv(keep):
                while len(pv_pending) > keep:
                    pv_pending.pop(0)()

            def emit_outproj_some(n):
                for _ in range(n):
                    if not op_pending:
                        return
                    try:
                        job = next(op_pending[0])
                    except StopIteration:
                        op_pending.pop(0)
                        continue
                    job()

            op_ready = []  # groups whose normalize is emitted
            for b in range(B):
                for g in range(NG):
                    gsl = slice(g * 512, (g + 1) * 512)
                    nkb = 4 * (g + 1)
                    pvs = [pvpool.tile([P, 512], f32, tag="pv",
                                       name=f"pv_{b}_{g}_{h}")
                           for h in range(2)]
                    for kb in range(nkb):
                        j = kb - 4 * g
                        col = b * NB + kb
                        qo = 128 * j if j > 0 else 0
                        sc2 = mnpool.tile([P, 2, 512], f32, tag="mn",
                                          name=f"sc2_{b}_{g}_{kb}")
                        for h in range(2):
                            hsl = slice(h * HD, (h + 1) * HD)
                            nc.tensor.matmul(
                                sc2[:, h, qo:512],
                                lhsT=kt_sb[hsl, b, kb * P:(kb + 1) * P],
                                rhs=qt_sb[hsl, b,
                                          g * 512 + qo:(g + 1) * 512],
                                start=True, stop=True)
                        pt = ptpool.tile([P, 2, 512], bf16, tag="pt")
                        if j >= 0:  # diagonal tile: additive causal mask
                            nc.vector.tensor_add(
                                sc2[:, :, qo:qo + P], sc2[:, :, qo:qo + P],
                                cm_sb[:])
                        nc.scalar.activation(pt[:, :, qo:512],
                                             sc2[:, :, qo:512], AF.Exp,
                                             bias=mb_sb[:, col:col + 1])

                        def pv_job(kb=kb, qo=qo, pt=pt, pvs=pvs, b=b,
                                   nkb=nkb):
                            for h in range(2):
                                nc.tensor.matmul(
                                    pvs[h][0:HD + 1, qo:512],
                                    lhsT=v_sb[:, h, b, kb, :],
                                    rhs=pt[:, h, qo:512],
                                    start=(kb == 0), stop=(kb == nkb - 1))
                        pv_pending.append(pv_job)
                        drain_pv(3)
                        if kb >= 1:
                            emit_outproj_some(1)

                    # group end: flush queued outproj slack, normalize chain
                    emit_outproj_some(1)

                    # Normalization is a 3-stage pipeline staggered one group
                    # per stage, so no engine-queue entry ever waits on a
                    # fresh producer (in-order queues head-of-line block
                    # otherwise): A) free pv psum via SBUF copy + DMA-shift
                    # the denom row to partition 0 (reciprocal_approx_fast
                    # only works at base 0); B) reciprocal + partition-
                    # broadcast; C) normalize muls (gpsimd) + at_sb DMA.
                    st = {}

                    def stageA(b=b, g=g, pvs=pvs, st=st):
                        st["pvsb"] = [
                            pvsbpool.tile([HD + 1, 512], f32, tag="pvsb",
                                          name=f"pvsb_{b}_{g}_{h}")
                            for h in range(2)]
                        for h in range(2):
                            nc.vector.tensor_copy(st["pvsb"][h][:],
                                                  pvs[h][0:HD + 1, :])
                        st["dn"] = recpool.tile([1, 2, 512], f32, tag="dn",
                                                name=f"dn_{b}_{g}")
                        for h in range(2):
                            nc.sync.dma_start(st["dn"][:, h, :],
                                              st["pvsb"][h][HD:HD + 1, :])

                    def stageB(b=b, g=g, st=st):
                        rec = recpool.tile([1, 2, 512], f32, tag="rec",
                                           name=f"rec_{b}_{g}")
                        nc.vector.reciprocal_approx_fast(
                            rec[:, :, :], st["dn"][:, :, :])
                        if DEBUG and b == 0 and g == 0:
                            nc.sync.dma_start(dp_d[0:HD + 1, :],
                                              st["pvsb"][0][:])
                            nc.sync.dma_start(dr_d[:], rec[:])
                        st["sx"] = []
                        for h in range(2):
                            sx = sxpool.tile([HD, 512], f32, tag="sx",
                                             name=f"sx_{b}_{g}_{h}")
                            nc.sync.dma_start(
                                sx[:],
                                rec[:, h, None, :]
                                .to_broadcast((1, HD, 512)))
                            st["sx"].append(sx)

                    def stageC(b=b, g=g, gsl=gsl, st=st):
                        nc.gpsimd.tensor_mul(
                            at_sb[0:HD, b, gsl],
                            st["pvsb"][0][0:HD, :], st["sx"][0][:])
                        # head 1 lands on partitions 64..127: engines cannot
                        # shift partitions -> mul to a tmp, DMA into place.
                        tmp = sxpool.tile([HD, 512], bf16, tag="tmp",
                                          name=f"tmp_{b}_{g}")
                        nc.gpsimd.tensor_mul(
                            tmp[:], st["pvsb"][1][0:HD, :], st["sx"][1][:])
                        nc.sync.dma_start(at_sb[HD:2 * HD, b, gsl], tmp[:])

                    # stage A goes through pv_pending (must follow the last
                    # PV of this group); B/C run at the next two group ends.
                    pv_pending.append(stageA)
                    if stageC_q:
                        stageC_q.pop(0)()        # C of group i-2
                    if stage_q:
                        b_job, c_job = stage_q.pop(0)
                        b_job()                  # B of group i-1
                        stageC_q.append(c_job)
                    stage_q.append((stageB, stageC))
                    op_ready.append((b, g))
                    if len(op_ready) >= 4:
                        op_pending.append(outproj_jobs(*op_ready.pop(0)))

            drain_pv(0)
            while stageC_q or stage_q:
                if stageC_q:
                    stageC_q.pop(0)()
                if stage_q:
                    b_job, c_job = stage_q.pop(0)
                    b_job()
                    stageC_q.append(c_job)
            for bg in op_ready:
                op_pending.append(outproj_jobs(*bg))
            emit_outproj_some(1000)

            if DEBUG:
                nc.sync.dma_start(dq_d[:], qt_sb[:, 0, 0:512])
                nc.sync.dma_start(dk_d[:], kt_sb[:, 0, 0:512])
                nc.sync.dma_start(dv_d[:], v_sb[:, :, 0, 0:4, :])
                nc.sync.dma_start(da_d[:], at_sb[:, 0, :])

    nc.compile()
    return nc


def _get_nc():
    if "nc" not in _CACHE:
        _CACHE["nc"] = _build_nc()
    return _CACHE["nc"]


def make_in_maps(x, attention_mask, Wq, bq, Wk, bk, Wv, bv, Wp, bp):
    """Host-side sharding: build the 8 per-core device input maps."""
    import ml_dtypes
    bf16 = ml_dtypes.bfloat16
    x = np.asarray(x, dtype=np.float32)
    scale = np.float32(1.0 / np.sqrt(HD))
    xT = np.ascontiguousarray(x.reshape(BS, D).T.astype(bf16))  # [D, BS]
    mb = (np.asarray(attention_mask).astype(np.float32) - 1.0) * np.float32(1e9)
    mb = np.ascontiguousarray(
        mb.reshape(B, NB, P).transpose(2, 0, 1).reshape(P, B * NB))
    # causal diagonal-tile mask (additive): 0 where q_local >= k_local,
    # else -1e9; duplicated for the two heads' strided slices.
    pp = np.arange(P)[:, None]
    ff = np.arange(P)[None, :]
    cm1 = np.where(ff >= pp, 0.0, -1e9).astype(np.float32)
    cm = np.ascontiguousarray(
        np.stack([cm1, cm1], axis=1))  # [128, 2, 128]

    Wq = (np.asarray(Wq, np.float32) * scale).astype(bf16)
    bq = np.asarray(bq, np.float32) * scale
    Wk = np.asarray(Wk, np.float32).astype(bf16)
    bk = np.asarray(bk, np.float32)
    Wv = np.asarray(Wv, np.float32).astype(bf16)
    bv = np.asarray(bv, np.float32)
    Wp = np.asarray(Wp, np.float32).astype(bf16)

    in_maps = []
    for c in range(NCORES):
        cs = slice(c * P, (c + 1) * P)
        in_maps.append({
            "xT": xT,
            "wq": np.ascontiguousarray(Wq[:, cs]),
            "wk": np.ascontiguousarray(Wk[:, cs]),
            "wv": np.ascontiguousarray(Wv[:, cs]),
            "bq": np.ascontiguousarray(bq[cs].reshape(P, 1)),
            "bk": np.ascontiguousarray(bk[cs].reshape(P, 1)),
            "bv": np.ascontiguousarray(bv[cs].reshape(P, 1)),
            "wp": np.ascontiguousarray(Wp[cs, :]),
            "maskb": mb,
            "cmask": cm,
        })
    return in_maps


def run(inputs, trace=False, tmpdir=None):
    """Compile (cached) + run on 8 cores. Returns (output, BassKernelResults)."""
    from concourse import bass_utils
    nc = _get_nc()
    in_maps = make_in_maps(**inputs)
    kwargs = {}
    if trace:
        kwargs = dict(trace=True, tmpdir=tmpdir)
    res = bass_utils.run_bass_kernel_spmd(
        nc, in_maps, core_ids=list(range(NCORES)), **kwargs)
    acc = np.zeros((BS, D), dtype=np.float64)
    for r in res.results:
        acc += r["yp"].astype(np.float64)
    out = (acc + np.asarray(inputs["bp"], np.float64)[None, :]).astype(
        np.float32)
    return out.reshape(B, S, D), res


def kernel(**inputs) -> np.ndarray:
    out, _ = run(inputs, trace=False)
    return out
